# revision 11
# baseline (speedup 1.0000x reference)
"""Trainium2 Bass kernel for ContrastiveAffinityLossWithMemoryV2.

Math: with MARGIN=4 and d = ||a-b|| <= 2 for unit vectors, relu(M-d) = M-d,
so each pairwise loss term simplifies:
    t*d^2 + (1-t)*(M-d)^2 = d^2 + (1-t)*(16 - 8*d)
Sum(d^2) and Sum(1-t) are *linear* and evaluated exactly on host from vector
sums; the only part needing the full B x B pair plane / B x C memory plane is
    P3 = Sum 8*d * (1-t)
which the device computes, sharded over 8 NeuronCores:
  - PE: psum = S via fp8 e4m3 DoubleRow matmuls (K=192 packed as 2 k-tiles of
    96 partitions; one matmul per 128x512 unit at 0.5 cycles/row)
  - ScalarE: d8 = sqrt(-128*psum + 128 + delta) = 8*d (negative scale folds
    the "2-2S" form into the activation's free affine; embeddings/bank rows
    are truncated toward zero in fp8 so every norm stays <= 1 and the sqrt
    argument stays positive)
  - VectorE: scalar_tensor_tensor fused multiply+reduce against host-shipped
    fp8 masks (stochastically rounded so quantization is unbiased), giving
    per-partition partial sums.

The pair plane is computed only for j > i. All cores run ONE program with a
fixed slot pattern; per-core differences live entirely in the data: each
core's znp tensor is [8 rhs windows x 512 cols | 6 lhs blocks x 128 cols]
where the host fills window w with a 512-col chunk and lhs slot p with a
128-row block of its choice. Cores 0-3 have chunk-count profile {8,7,2,1},
cores 4-7 {6,5,4,3}; both embed exactly into the fixed pattern whose
window slot counts are {4,3,2,2,2,2,2,1}.  Host combines device partials
with the closed-form terms.
"""

import numpy as np
import ml_dtypes

N_CLASSES = 8192
B = 4096
D = 192  # 256 * 0.75
KP = 96  # partitions per k-tile (2 tiles of 96 = 192)
NCORES = 8
MARGIN = 4.0
MEMORY_WEIGHT = 0.5
WARMUP_STEPS = 1000
MOM_WARMUP = 5000
BASE_MOM = 0.9
BG_SIM = 0.2
BG_OTHER_SIM = 0.01
EPS = 1e-12
DELTA2 = 0.01
NGU = 18                    # G-plane units per core (144 / 8)
OFFLOAD_GP = False          # run one G group's mask-reduce on GPSIMD
GW = 8 * 512                # G-rhs window region width in znp
ZNP_COLS = GW + 6 * 128     # + 6 lhs block slots

bf16 = ml_dtypes.bfloat16
f8 = ml_dtypes.float8_e4m3

# row-block deal: cores 0-3 get chunk-counts {8,7,2,1}, cores 4-7 {6,5,4,3}
CORE_RBS = [[k, 4 + k, 24 + k, 28 + k] for k in range(4)] + \
           [[8 + k, 12 + k, 16 + k, 20 + k] for k in range(4)]

# fixed G-plane slot pattern: (window, lhs slot), ordered so consecutive
# slots share the same lhs block (stationary reuse on the PE)
P_SLOTS = [
    (0, 0), (1, 0), (2, 0), (4, 0), (6, 0), (7, 0),
    (0, 1), (1, 1), (2, 1), (4, 1), (6, 1),
    (0, 2), (1, 2),
    (0, 3),
    (3, 4), (5, 4),
    (3, 5), (5, 5),
]
# per-core window contents (chunk id per window) and lhs slot contents
# (index into CORE_RBS[core] per lhs slot):
W_A = [7, 6, 1, 2, 3, 4, 5, 0]       # cores 0-3
W_B = [5, 4, 6, 6, 7, 7, 3, 2]       # cores 4-7
LHS_A = [0, 1, 2, 3, 0, 1]
LHS_B = [0, 1, 2, 3, 2, 3]


def _g_chunks(rb):
    """512-col chunks containing any j > i for row-block rb."""
    return [cc for cc in range(8) if 512 * cc + 511 >= 128 * rb + 1]


def _core_gunits(core):
    """(rb, cc) per G slot for this core; asserts global exact cover."""
    rbs = CORE_RBS[core]
    W = W_A if core < 4 else W_B
    LHS = LHS_A if core < 4 else LHS_B
    return [(rbs[LHS[lp]], W[w]) for (w, lp) in P_SLOTS]


def _check_cover():
    seen = []
    for core in range(NCORES):
        seen += _core_gunits(core)
    need = [(rb, cc) for rb in range(32) for cc in _g_chunks(rb)]
    assert sorted(seen) == sorted(need), "G-plane cover mismatch"


_check_cover()

_CACHE = {}


def cap_fp8(v):
    """fp32 -> fp8 e4m3 truncated toward zero: row L2 norms can only shrink."""
    x = np.ascontiguousarray(v, dtype=np.float32)
    y = x.astype(f8)
    yb = y.view(np.uint8).copy()
    over = np.abs(y.astype(np.float32)) > np.abs(x)
    yb[over & ((yb & 0x7F) > 0)] -= 1
    return yb.view(f8)


def stoch_fp8(v, seed):
    """Stochastic rounding to float8_e4m3 (values >= 0)."""
    x = np.ascontiguousarray(v, dtype=np.float32)
    y = x.astype(f8)
    yb = y.view(np.uint8).copy()
    over = np.abs(y.astype(np.float32)) > x
    yb[over & ((yb & 0x7F) > 0)] -= 1
    fl = yb.view(f8)
    ce = (yb + (fl.astype(np.float32) < x).astype(np.uint8)).view(f8)
    flf = fl.astype(np.float32)
    gap = ce.astype(np.float32) - flf
    p = np.where(gap > 0, (x - flf) / np.where(gap > 0, gap, 1.0), 0.0)
    rng = np.random.default_rng(seed)
    up = rng.random(x.shape, dtype=np.float32) < p
    return np.where(up, ce, fl).astype(f8)


def _bank_chains(zn, y_true, momentum):
    """Replicate the reference's sequential per-sample EMA scatter (fp32)."""
    valid = (y_true >= 0) & (y_true < N_CLASSES)
    lc = np.clip(y_true, 0, N_CLASSES - 1)
    m = np.float32(momentum)
    one_m = np.float32(1.0 - momentum)
    bank = {}
    for i in np.nonzero(valid)[0]:
        c = int(lc[i])
        if c not in bank:
            bank[c] = zn[i].copy()
        else:
            ema = m * bank[c] + one_m * zn[i]
            n = np.float32(np.sqrt(np.float32((ema ** 2).sum())))
            bank[c] = ema / max(n, np.float32(EPS))
    return bank


def _groups(CS):
    """Group structure: list of lists of units.

    Units: ("s", lhs slot, bank chunk) | ("g", lhs slot, window).
    A single 1-unit prologue group starts the ACT/DVE pipeline early; all
    other groups are 2048 wide (uniform widths keep the psum double-buffer
    refill fully hidden behind the previous group's activation).
    """
    s_units = [("s", ib, cc) for ib in range(4) for cc in range(CS)]
    g_units = [("g", lp, w) for (w, lp) in P_SLOTS]
    groups = [s_units[0:1]]
    i = 1
    while i < len(s_units):
        groups.append(s_units[i:i + 4])
        i += 4
    n_groups_s = len(groups)
    for q in range(0, NGU, 4):
        groups.append(g_units[q:q + 4])
    return groups, n_groups_s


def _build_nc(CS):
    """CS = number of 512-wide S-plane chunks (CP = 512*CS classes)."""
    from concourse import bacc, tile, mybir

    dt = mybir.dt
    CP = 512 * CS
    nc = bacc.Bacc("TRN2", target_bir_lowering=False, debug=False)

    lhp_d = nc.dram_tensor("lhp", (KP, 2, 6, 128), dt.float8e4, kind="ExternalInput")
    znw_d = nc.dram_tensor("znw", (KP, 8, 2, 512), dt.float8e4, kind="ExternalInput")
    bkp_d = nc.dram_tensor("bkp", (KP, CS, 2, 512), dt.float8e4, kind="ExternalInput")
    r1_d = nc.dram_tensor("r1", (128, 4 * CP), dt.float8e4, kind="ExternalInput")
    t2_d = nc.dram_tensor("t2", (128, NGU * 512), dt.float8e4, kind="ExternalInput")
    out_d = nc.dram_tensor("acc_out", (128, 16), dt.float32, kind="ExternalOutput")

    groups, n_groups_s = _groups(CS)
    n_groups = len(groups)
    assert n_groups <= 16

    with tile.TileContext(nc) as tc:
        with (
            tc.tile_pool(name="const", bufs=1) as constp,
            tc.tile_pool(name="d8p", bufs=3) as d8p,
            tc.tile_pool(name="ep", bufs=2) as ep,
            tc.tile_pool(name="psp", bufs=2, space="PSUM") as psp,
        ):
            # resident operands; [KP, 2, N] with k-tile index in free dim 0
            lhp = constp.tile([KP, 2, 6, 128], dt.float8e4, tag="lhp")
            znw = constp.tile([KP, 8, 2, 512], dt.float8e4, tag="znw")
            bkp = constp.tile([KP, CS, 2, 512], dt.float8e4, tag="bkp")
            r1 = constp.tile([128, 4 * CP], dt.float8e4, tag="r1")
            t2 = constp.tile([128, NGU * 512], dt.float8e4, tag="t2")

            bias_t = constp.tile([128, 1], dt.float32)
            nc.gpsimd.memset(bias_t[:], 128.0 + float(DELTA2))
            warm = constp.tile([128, 1], dt.float32)
            nc.gpsimd.memset(warm[:], 1.0)

            acc_all = constp.tile([128, 16], dt.float32)
            nc.gpsimd.memset(acc_all[:], 0.0)

            # Scalar issues nothing before its ACT table loads; the
            # warmup activation below triggers their insertion right here.
            nc.scalar.activation(
                warm[:], warm[:], mybir.ActivationFunctionType.Sqrt, scale=1.0,
            )

            def r1_sl(eng, a, b):
                a, b = min(a, 4 * CP), min(b, 4 * CP)
                if a < b:
                    eng.dma_start(r1[:, a:b], r1_d[:, a:b])

            # GpSimd fresh queues: group-0 operands + late bank chunks
            nc.gpsimd.dma_start(bkp[:, 0, 0], bkp_d[:, 0, 0])
            nc.gpsimd.dma_start(bkp[:, 0, 1], bkp_d[:, 0, 1])
            if CS > 3:
                nc.gpsimd.dma_start(bkp[:, 3, 0], bkp_d[:, 3, 0])
                nc.gpsimd.dma_start(bkp[:, 3, 1], bkp_d[:, 3, 1])
            if CS > 4:
                nc.gpsimd.dma_start(bkp[:, 4, 0], bkp_d[:, 4, 0])
                nc.gpsimd.dma_start(bkp[:, 4, 1], bkp_d[:, 4, 1])
            if CS > 5:
                nc.gpsimd.dma_start(bkp[:, 5, 0], bkp_d[:, 5, 0])
                nc.gpsimd.dma_start(bkp[:, 5, 1], bkp_d[:, 5, 1])
            if CS > 6:
                nc.gpsimd.dma_start(bkp[:, 6:CS], bkp_d[:, 6:CS])
            r1_sl(nc.gpsimd, 0, 512)

            # Sync fresh queues: lhs blocks, group-1 operands
            nc.sync.dma_start(lhp[:, 0], lhp_d[:, 0])
            nc.sync.dma_start(lhp[:, 1], lhp_d[:, 1])
            if CS > 1:
                nc.sync.dma_start(bkp[:, 1, 0], bkp_d[:, 1, 0])
                nc.sync.dma_start(bkp[:, 1, 1], bkp_d[:, 1, 1])
            if CS > 2:
                nc.sync.dma_start(bkp[:, 2, 0], bkp_d[:, 2, 0])
                nc.sync.dma_start(bkp[:, 2, 1], bkp_d[:, 2, 1])
            r1_sl(nc.sync, 512, 1536)
            r1_sl(nc.sync, 1536, 2560)
            # Sync reuse wave: remaining masks + G-plane operands in
            # consumption order
            r1_sl(nc.sync, 2560, 3584)
            r1_sl(nc.sync, 3584, 4608)
            r1_sl(nc.sync, 4608, 5632)
            r1_sl(nc.sync, 5632, 6656)
            r1_sl(nc.sync, 6656, 7680)
            r1_sl(nc.sync, 7680, 8704)
            r1_sl(nc.sync, 8704, 9728)
            r1_sl(nc.sync, 9728, 10752)
            r1_sl(nc.sync, 10752, 4 * CP)
            nc.sync.dma_start(znw[:, 0:1], znw_d[:, 0:1])
            nc.sync.dma_start(znw[:, 1:2], znw_d[:, 1:2])
            nc.sync.dma_start(znw[:, 2:3], znw_d[:, 2:3])
            nc.sync.dma_start(znw[:, 3:4], znw_d[:, 3:4])
            nc.sync.dma_start(znw[:, 4:5], znw_d[:, 4:5])
            nc.sync.dma_start(znw[:, 5:6], znw_d[:, 5:6])
            nc.sync.dma_start(znw[:, 6:7], znw_d[:, 6:7])
            nc.sync.dma_start(znw[:, 7:8], znw_d[:, 7:8])
            nc.sync.dma_start(t2[:, 0:1024], t2_d[:, 0:1024])
            nc.sync.dma_start(t2[:, 1024:2048], t2_d[:, 1024:2048])
            nc.sync.dma_start(t2[:, 2048:3072], t2_d[:, 2048:3072])
            nc.sync.dma_start(t2[:, 3072:4096], t2_d[:, 3072:4096])
            nc.sync.dma_start(t2[:, 4096:6144], t2_d[:, 4096:6144])
            nc.sync.dma_start(t2[:, 6144:8192], t2_d[:, 6144:8192])
            nc.sync.dma_start(t2[:, 8192:NGU * 512], t2_d[:, 8192:NGU * 512])

            # one G group's mask-reduce runs on the (otherwise idle) GPSIMD
            # engine; dedicated tiles so its slower pace never stalls the
            # d8/et rings used by the DVE groups.
            off_gi = n_groups_s if OFFLOAD_GP else -1
            d8x = constp.tile([128, 2048], dt.bfloat16, tag="d8x")
            etx = constp.tile([128, 2048], dt.bfloat16, tag="etx")

            s_off = 0
            g_off = 0
            for gi in range(n_groups):
                gunits = groups[gi]
                gw = 512 * len(gunits)
                ps = psp.tile([128, 2048], dt.float32, tag="ps")
                for q, (kind, lp, w) in enumerate(gunits):
                    lhs = lhp[:, :, lp]
                    rhs = bkp[:, w] if kind == "s" else znw[:, w]
                    nc.tensor.matmul(
                        ps[:, q * 512:(q + 1) * 512],
                        lhs, rhs,
                        start=True, stop=True,
                        perf_mode=mybir.MatmulPerfMode.DoubleRow,
                    )
                d8 = d8x if gi == off_gi else d8p.tile([128, 2048], dt.bfloat16, tag="d8")
                nc.scalar.activation(
                    d8[:, 0:gw], ps[:, 0:gw],
                    mybir.ActivationFunctionType.Sqrt,
                    bias=bias_t[:], scale=-128.0,
                )
                if gunits[0][0] == "s":
                    mask, mc0 = r1, s_off
                    s_off += gw
                else:
                    mask, mc0 = t2, g_off
                    g_off += gw
                if gi == off_gi:
                    nc.gpsimd.tensor_tensor(
                        etx[:, 0:gw], d8[:, 0:gw], mask[:, mc0:mc0 + gw],
                        mybir.AluOpType.mult,
                    )
                    nc.gpsimd.tensor_reduce(
                        out=acc_all[0:1, 15:16], in_=etx[:, 0:gw],
                        axis=mybir.AxisListType.XYZWC, op=mybir.AluOpType.add,
                    )
                    continue
                et = ep.tile([128, 2048], dt.bfloat16, tag="et")
                nc.vector.scalar_tensor_tensor(
                    out=et[:, 0:gw],
                    in0=d8[:, 0:gw],
                    scalar=1.0,
                    in1=mask[:, mc0:mc0 + gw],
                    op0=mybir.AluOpType.mult,
                    op1=mybir.AluOpType.mult,
                    accum_out=acc_all[:, gi:gi + 1],
                )

            nc.sync.dma_start(out_d[:], acc_all[:])

    nc.compile()
    return nc, n_groups, n_groups_s


def _get_nc(CS):
    key = ("nc3", CS)
    if key not in _CACHE:
        _CACHE[key] = _build_nc(CS)
    return _CACHE[key]


def kernel(y_true, y_pred, lookup, global_step, current_epoch, _want_trace=False):
    from concourse.bass_utils import run_bass_kernel_spmd

    y_true = np.asarray(y_true).astype(np.int64)
    y_pred = np.asarray(y_pred, dtype=np.float32)
    lookup = np.asarray(lookup, dtype=np.float32)
    gs = int(np.asarray(global_step))

    if gs < MOM_WARMUP:
        momentum = 0.5 + (BASE_MOM - 0.5) * (gs / MOM_WARMUP)
    else:
        momentum = BASE_MOM
    progress = min(1.0, (gs - WARMUP_STEPS) / 5000.0)
    aw = MEMORY_WEIGHT * progress

    # ---- host: normalize, bank scatter-EMA, compaction ----
    z = y_pred[:, :D]
    nrm = np.sqrt((z.astype(np.float64) ** 2).sum(axis=1))
    zn = (z / np.maximum(nrm, EPS)[:, None]).astype(np.float32)

    valid = (y_true >= 0) & (y_true < N_CLASSES)
    bg = ~valid
    nv = int(valid.sum())
    lc = np.clip(y_true, 0, N_CLASSES - 1)

    bank = _bank_chains(zn, y_true, momentum)
    init_list = np.array(sorted(bank.keys()), dtype=np.int64)
    C = len(init_list)
    CS = max(1, (C + 511) // 512)
    CP = 512 * CS

    zn_q = cap_fp8(zn)                                      # (B, D)
    bank_rows = (
        np.stack([bank[c] for c in init_list])
        if C else np.zeros((0, D), np.float32)
    )
    bank_q = cap_fp8(bank_rows)                             # (C, D)

    znd = zn_q.astype(np.float64)
    bankd = bank_q.astype(np.float64)

    # ---- host: exact linear terms (fp64) ----
    R = lookup[lc]                    # (B, 8192)
    R_init = R[:, init_list]          # (B, C)
    A_S = 2.0 * nv * C - 2.0 * float(znd[valid].sum(0) @ bankd.sum(0))
    B_S = nv * C - float(R_init[valid].sum(dtype=np.float64))

    T_up = R[:, lc]                   # (B, B): lookup[lc_i, lc_j]
    both_bg = bg[:, None] & bg[None, :]
    one_bg = bg[:, None] ^ bg[None, :]
    T_up = np.where(both_bg, np.float32(BG_SIM),
                    np.where(one_bg, np.float32(BG_OTHER_SIM), T_up))
    # upper-triangle (i<j) oriented pair targets; zero elsewhere
    T_up = np.triu(T_up, 1)

    Np = B * (B - 1) // 2
    szn = znd.sum(0)
    sumG_offdiag = float(szn @ szn) - float((znd ** 2).sum())
    A_G = 2.0 * Np - sumG_offdiag
    B_G = Np - float(T_up.sum(dtype=np.float64))

    # ---- device operand construction ----
    znT = np.ascontiguousarray(zn_q.T)                      # (192, B) fp8
    bankT = np.zeros((D, CP), dtype=f8)
    if C:
        bankT[:, 0:C] = bank_q.T

    bkp_all = np.empty((KP, CS, 2, 512), dtype=f8)
    for c in range(CS):
        bkp_all[:, c, 0, :] = bankT[0:KP, c * 512:(c + 1) * 512]
        bkp_all[:, c, 1, :] = bankT[KP:D, c * 512:(c + 1) * 512]

    in_maps = []
    for core in range(NCORES):
        rbs = CORE_RBS[core]
        W = W_A if core < 4 else W_B
        LHS = LHS_A if core < 4 else LHS_B

        znwc = np.empty((KP, 8, 2, 512), dtype=f8)
        for w in range(8):
            cc = W[w]
            znwc[:, w, 0, :] = znT[0:KP, cc * 512:(cc + 1) * 512]
            znwc[:, w, 1, :] = znT[KP:D, cc * 512:(cc + 1) * 512]
        lhpc = np.empty((KP, 2, 6, 128), dtype=f8)
        for lp in range(6):
            rb = rbs[LHS[lp]]
            lhpc[:, 0, lp, :] = znT[0:KP, rb * 128:rb * 128 + 128]
            lhpc[:, 1, lp, :] = znT[KP:D, rb * 128:rb * 128 + 128]

        r1 = np.zeros((128, 4 * CP), dtype=f8)
        for ib, rb in enumerate(rbs):
            rr = slice(rb * 128, rb * 128 + 128)
            m = (1.0 - R_init[rr]) * valid[rr, None]        # (128, C)
            r1[:, ib * CP:ib * CP + C] = stoch_fp8(m, seed=1000 + rb)

        t2 = np.zeros((128, NGU * 512), dtype=f8)
        for u, (rb, cc) in enumerate(_core_gunits(core)):
            blk = 1.0 - T_up[rb * 128:rb * 128 + 128, cc * 512:(cc + 1) * 512]
            jj = np.arange(cc * 512, cc * 512 + 512)[None, :]
            ii = np.arange(rb * 128, rb * 128 + 128)[:, None]
            blk = np.where(jj > ii, blk, 0.0)
            t2[:, u * 512:(u + 1) * 512] = stoch_fp8(blk, seed=2000 + rb * 8 + cc)

        in_maps.append({
            "lhp": lhpc,
            "znw": znwc,
            "bkp": bkp_all,
            "r1": r1,
            "t2": t2,
        })

    nc, n_groups, n_groups_s = _get_nc(CS)
    if _want_trace:
        import tempfile
        try:
            from trn_agent_boot.trn_boot import _ntff_profile_via_ctypes
            hook = _ntff_profile_via_ctypes("/opt/axon/libaxon_pjrt.so")
            outdir = tempfile.mkdtemp(prefix="ntff_")
            with hook(outdir, [0]):
                res = run_bass_kernel_spmd(nc, in_maps, list(range(NCORES)))
            _CACHE["last_profile_dir"] = outdir
        except Exception as e:
            _CACHE["trace_error"] = repr(e)
            res = run_bass_kernel_spmd(nc, in_maps, list(range(NCORES)))
        _CACHE["last_results"] = res
    else:
        res = run_bass_kernel_spmd(nc, in_maps, list(range(NCORES)))

    P3S = 0.0
    P3G = 0.0
    for r in res.results:
        acc = np.asarray(r["acc_out"], dtype=np.float64)
        P3S += float(acc[:, 0:n_groups_s].sum())
        P3G += float(acc[:, n_groups_s:n_groups].sum()) + float(acc[0, 15])

    mem_sum = A_S + 16.0 * B_S - P3S
    denom = max(nv * C, 1)
    mem_loss = mem_sum / denom

    batch_sum = A_G + 16.0 * B_G - P3G
    batch_loss = batch_sum / Np

    loss = (1.0 - aw) * batch_loss + aw * mem_loss
    return np.float32(loss)

